# revision 26
# baseline (speedup 1.0000x reference)
"""SO-Net encoder kernel for 8 NeuronCores (Bass/Tile), data-parallel over batch."""

import numpy as np
import ml_dtypes

import concourse.bass as bass
import concourse.mybir as mybir
import concourse.tile as tile
from concourse import bacc
from concourse.bass_utils import run_bass_kernel_spmd
from concourse.masks import make_identity

F32 = mybir.dt.float32
F32R = mybir.dt.float32r
BF16 = mybir.dt.bfloat16
U16 = mybir.dt.uint16
I32 = mybir.dt.int32
AF = mybir.ActivationFunctionType
OP = mybir.AluOpType

B, N, M, K = 8, 4096, 256, 3
NCH = N // 128          # 32 point chunks
NB = 4                  # gather blocks (cores 0,2,4,6), 1024 points each
BLK = N // NB           # 1024

_CACHE = {}


def _build():
    nc = bacc.Bacc("TRN2", target_bir_lowering=False)

    # ---------------- DRAM tensors ----------------
    x_d = nc.dram_tensor("x4", [4, N], F32, kind="ExternalInput")
    node2_d = nc.dram_tensor("node2", [4, M], F32, kind="ExternalInput")
    w1_d = nc.dram_tensor("w1", [4, 64], BF16, kind="ExternalInput")     # [fpw0.T; b0]
    w2_d = nc.dram_tensor("w2", [65, 128], BF16, kind="ExternalInput")   # [fpw1.T; b1]
    w3_d = nc.dram_tensor("w3", [128, 256], BF16, kind="ExternalInput")  # fpw2.T
    b3_d = nc.dram_tensor("b3", [128, 2], F32, kind="ExternalInput")     # fp_b2 chunks
    w4a_d = nc.dram_tensor("w4a", [256, 384], BF16, kind="ExternalInput")  # fpw3.T[:256]
    w4b_d = nc.dram_tensor("w4b", [4, 384], BF16, kind="ExternalInput")    # [fpw3.T[256:259]; b3]
    g1_d = nc.dram_tensor("g1", [388, 512], BF16, kind="ExternalInput")
    g2_d = nc.dram_tensor("g2", [512, 512], BF16, kind="ExternalInput")
    bg2_d = nc.dram_tensor("bg2", [128, 4], F32, kind="ExternalInput")
    g3_d = nc.dram_tensor("g3", [512, 768], BF16, kind="ExternalInput")
    bg3_d = nc.dram_tensor("bg3", [128, 6], F32, kind="ExternalInput")
    g4_d = nc.dram_tensor("g4", [1156, 1024], BF16, kind="ExternalInput")
    out_d = nc.dram_tensor("out", [1024], F32, kind="ExternalOutput")
    stage_d = nc.dram_tensor("stage", [3 * N], BF16)  # scratch: flat idx rows

    with tile.TileContext(nc) as tc:
        with (
            tc.tile_pool(name="per", bufs=1) as per,          # persistent sbuf
            tc.tile_pool(name="tr", bufs=5) as tr,            # transient sbuf
            tc.tile_pool(name="tr2", bufs=2) as tr2,          # big transients
            tc.tile_pool(name="pp", bufs=1, space="PSUM") as pp,   # persistent psum
            tc.tile_pool(name="pt", bufs=4, space="PSUM") as pt,   # rolling psum
        ):
            # ------------- constants -------------
            ident = per.tile([128, 128], F32)
            make_identity(nc, ident[:])
            io = per.tile([128, 256], I32)
            nc.gpsimd.iota(io[:], pattern=[[1, 256]], base=0, channel_multiplier=0)
            iota256 = per.tile([128, 256], BF16)
            nc.vector.tensor_copy(iota256[:], io[:])
            e4i = per.tile([4, 32, 4], I32)
            nc.gpsimd.iota(e4i[:], pattern=[[0, 32], [1, 4]], base=0,
                           channel_multiplier=-1)
            e4f = per.tile([4, 128], F32)
            nc.vector.tensor_copy(e4f[:], e4i[:].rearrange("c a b -> c (a b)"))
            e4 = per.tile([4, 128], F32)
            nc.vector.tensor_scalar(out=e4[:], in0=e4f[:], scalar1=0.0, scalar2=None,
                                    op0=OP.is_equal)
            ones_col = per.tile([1, 128], F32)
            nc.vector.memset(ones_col[:], 1.0)
            sel4 = per.tile([4, 1], F32)
            nc.vector.tensor_scalar(out=sel4[:], in0=e4f[:, 3:4], scalar1=0.0,
                                    scalar2=None, op0=OP.is_equal)

            # ------------- weights to SBUF -------------
            w1 = per.tile([128, 64], BF16)      # replicated at 32-offsets
            w4b = per.tile([128, 384], BF16)
            for m in range(4):
                nc.sync.dma_start(w1[32 * m:32 * m + 4, :], w1_d[:])
                nc.sync.dma_start(w4b[32 * m:32 * m + 4, :], w4b_d[:])
            w2 = per.tile([65, 128], BF16)
            nc.sync.dma_start(w2[:], w2_d[:])
            w3 = per.tile([128, 256], BF16)
            nc.sync.dma_start(w3[:], w3_d[:])
            b3 = per.tile([128, 2], F32)
            nc.sync.dma_start(b3[:], b3_d[:])
            g1 = per.tile([128, 4, 512], BF16)   # chunks 0..2 = 128 rows, chunk 3 rows 0..3
            g2 = per.tile([128, 4, 512], BF16)
            g3 = per.tile([128, 4, 768], BF16)
            g4 = per.tile([128, 10, 1024], BF16)
            bg2 = per.tile([128, 4], F32)
            bg3 = per.tile([128, 6], F32)
            nc.sync.dma_start(bg2[:], bg2_d[:])
            nc.sync.dma_start(bg3[:], bg3_d[:])
            for c in range(3):
                nc.sync.dma_start(g1[:, c, :], g1_d[128 * c:128 * (c + 1), :])
            nc.sync.dma_start(g1[0:4, 3, :], g1_d[384:388, :])
            for c in range(4):
                nc.sync.dma_start(g2[:, c, :], g2_d[128 * c:128 * (c + 1), :])
                nc.sync.dma_start(g3[:, c, :], g3_d[128 * c:128 * (c + 1), :])
            for c in range(9):
                nc.sync.dma_start(g4[:, c, :], g4_d[128 * c:128 * (c + 1), :])
            nc.sync.dma_start(g4[0:4, 9, :], g4_d[1152:1156, :])
            w4a_t = per.tile([128, 2, 384], BF16)
            for c in range(2):
                nc.sync.dma_start(w4a_t[:, c, :], w4a_d[128 * c:128 * (c + 1), :])

            # ------------- x / node prep -------------
            x_sb = per.tile([4, N], F32)
            nc.sync.dma_start(x_sb[:], x_d[:])
            node2 = per.tile([4, M], F32)
            nc.sync.dma_start(node2[:], node2_d[:])

            # x_rep [128, N] f32: row p = x_sb[p%4]
            x_rep = per.tile([128, N], F32)
            for j in range(8):
                xr_ps = pt.tile([128, 512], F32, tag="roll")
                nc.tensor.matmul(xr_ps[:], e4[:], x_sb[:, 512 * j:512 * (j + 1)],
                                 start=True, stop=True)
                nc.scalar.copy(x_rep[:, 512 * j:512 * (j + 1)], xr_ps[:])

            # xT (bf16) [128, 4*NCH]: chunk t cols 4t..4t+4 = x_ext.T rows
            xt_ps = pt.tile([128, 128], F32, tag="roll")
            for t in range(NCH):
                nc.tensor.transpose(xt_ps[:, 4 * t:4 * (t + 1)],
                                    x_sb[:, 128 * t:128 * (t + 1)], ident[0:4, 0:4])
            xT = per.tile([128, 128], BF16)
            nc.vector.tensor_copy(xT[:], xt_ps[:])

            # ------------- P1: scores, top-3, masks, xsums -------------
            masks = per.tile([128, 96, 256], BF16)
            idxflat = per.tile([3, N], BF16)
            sums_ps = pp.tile([4, 256], F32)
            for t in range(NCH):
                s_ps = pt.tile([128, 256], F32, tag="roll")
                nc.tensor.matmul(s_ps[:], x_sb[:, 128 * t:128 * (t + 1)], node2[:],
                                 start=True, stop=True)
                s_sb = tr.tile([128, 256], F32, tag="s_sb")
                nc.scalar.copy(s_sb[:], s_ps[:])
                mx8 = tr.tile([128, 8], F32, tag="mx8")
                nc.vector.max(out=mx8[:], in_=s_sb[:])
                idx8 = tr.tile([128, 8], U16, tag="idx8")
                nc.vector.max_index(out=idx8[:], in_max=mx8[:], in_values=s_sb[:])
                idxf = tr.tile([128, 8], F32, tag="idxf")
                nc.scalar.copy(idxf[:], idx8[:])
                # masks for k=0,1,2 (split across DVE and GpSimd)
                meng = nc.vector if t % 2 == 0 else nc.gpsimd
                for k in range(3):
                    meng.tensor_scalar(out=masks[:, 32 * k + t, :], in0=iota256[:],
                                       scalar1=idxf[:, k:k + 1], scalar2=None,
                                       op0=OP.is_equal)
                    nc.tensor.matmul(sums_ps[:], xT[:, 4 * t:4 * (t + 1)],
                                     masks[:, 32 * k + t, :],
                                     start=(t == 0 and k == 0),
                                     stop=(t == NCH - 1 and k == 2))
                # transpose idx -> flat (pack 4 chunks per psum tile)
                if t % 4 == 0:
                    tp_ps = pt.tile([8, 512], F32, tag="roll")
                nc.tensor.transpose(tp_ps[:, 128 * (t % 4):128 * (t % 4 + 1)],
                                    idxf[:], ident[:])
                if t % 4 == 3:
                    nc.scalar.copy(idxflat[:, 128 * (t - 3):128 * (t + 1)], tp_ps[0:3, :])

            # ------------- P1.5: mean tables -------------
            sums_sb = per.tile([4, 256], F32)
            nc.vector.tensor_copy(sums_sb[:], sums_ps[:])
            cnt_ps = pt.tile([1, 256], F32, tag="roll")
            nc.tensor.matmul(cnt_ps[:], sel4[:], sums_sb[:], start=True, stop=True)
            cnt_row = per.tile([1, 256], F32)
            nc.vector.tensor_copy(cnt_row[:], cnt_ps[:])
            rec = per.tile([1, 256], F32)
            nc.vector.tensor_scalar_add(rec[:], cnt_row[:], 1e-5)
            nc.vector.reciprocal(rec[:], rec[:])
            rr_ps = pt.tile([4, 256], F32, tag="roll")
            nc.tensor.matmul(rr_ps[:], ones_col[0:1, 0:4], rec[:], start=True, stop=True)
            rec4 = per.tile([4, 256], F32)
            nc.vector.tensor_copy(rec4[:], rr_ps[:])
            mean_sb = per.tile([4, 256], F32)
            nc.vector.memset(mean_sb[:], 0.0)
            nc.vector.tensor_tensor(out=mean_sb[0:3, :], in0=sums_sb[0:3, :],
                                    in1=rec4[0:3, :], op=OP.mult)
            mr_ps = pt.tile([128, 256], F32, tag="roll")
            nc.tensor.matmul(mr_ps[:], e4[:], mean_sb[:], start=True, stop=True)
            mean_rep = per.tile([128, 256], F32)
            nc.vector.tensor_copy(mean_rep[:], mr_ps[:])
            mean_bf = per.tile([4, 256], BF16)
            nc.vector.memset(mean_bf[:], 1.0)
            nc.vector.tensor_copy(mean_bf[0:3, :], mean_sb[0:3, :])
            hp_ps = pt.tile([128, 256], F32, tag="roll")
            nc.tensor.matmul(hp_ps[:], ones_col[:], cnt_row[:], start=True, stop=True)
            hasp = per.tile([128, 256], I32)
            nc.vector.tensor_copy(hasp[:], hp_ps[:])
            nc.sync.dma_start(stage_d[:], idxflat[:])

            # ------------- P2: per-k resnet1 + accumulation -------------
            acc = [pp.tile([128, 256], F32, tag=f"acc{c}", name=f"acc{c}") for c in range(3)]
            fb_row = per.tile([1, 384], F32)
            h1 = per.tile([65, N], BF16)
            nc.vector.memset(h1[64:65, :], 1.0)
            h2 = per.tile([128, N], BF16)
            h3a = per.tile([128, N], BF16)
            h3b = per.tile([128, N], BF16)
            for k in range(3):
                wrapf = tr.tile([128, BLK // 16], BF16, tag="wrapf")
                nc.vector.memset(wrapf[:], 0.0)
                for m in range(NB):
                    nc.sync.dma_start(
                        wrapf[32 * m:32 * m + 16, :],
                        bass.AP(stage_d, k * N + BLK * m, [[1, 16], [16, BLK // 16]]))
                wrap = tr.tile([128, BLK // 16], U16, tag="wrap")
                nc.vector.tensor_copy(wrap[:], wrapf[:])
                ctr = tr2.tile([128, BLK, 1], F32, tag="ctr")
                nc.gpsimd.indirect_copy(ctr[:], mean_rep[:].rearrange("p (a b) -> p a b", b=1),
                                        wrap[:], True)
                xdec = tr2.tile([128, BLK], BF16, tag="xdec")
                for m in range(NB):
                    nc.vector.tensor_tensor(
                        out=xdec[32 * m:32 * m + 16, :],
                        in0=x_rep[32 * m:32 * m + 16, BLK * m:BLK * (m + 1)],
                        in1=ctr[32 * m:32 * m + 16, :, 0], op=OP.subtract)
                # L1: [64, N] bf16 (+ones row)
                for m in range(NB):
                    for h in range(2):
                        p1 = pt.tile([64, 512], F32, tag="roll")
                        nc.tensor.matmul(p1[:], w1[32 * m:32 * m + 4, :],
                                         xdec[32 * m:32 * m + 4, 512 * h:512 * (h + 1)],
                                         start=True, stop=True,
                                         tile_position=(32 * m, 0))
                        if (2 * m + h) % 2 == 0:
                            nc.scalar.activation(h1[0:64, BLK * m + 512 * h:BLK * m + 512 * (h + 1)],
                                                 p1[:], AF.Relu)
                        else:
                            nc.vector.tensor_scalar_max(
                                h1[0:64, BLK * m + 512 * h:BLK * m + 512 * (h + 1)],
                                p1[:], 0.0)
                # L2: [128, N]
                for j in range(8):
                    p2 = pt.tile([128, 512], F32, tag="roll")
                    nc.tensor.matmul(p2[:], w2[:], h1[:, 512 * j:512 * (j + 1)],
                                     start=True, stop=True)
                    if j % 2 == 0:
                        nc.scalar.activation(h2[:, 512 * j:512 * (j + 1)], p2[:], AF.Relu)
                    else:
                        nc.vector.tensor_scalar_max(h2[:, 512 * j:512 * (j + 1)], p2[:], 0.0)
                # L3: [256 -> 2 tiles, N], bias via ACT
                for c, h3t in enumerate([h3a, h3b]):
                    for j in range(8):
                        p3 = pt.tile([128, 512], F32, tag="roll")
                        nc.tensor.matmul(p3[:], w3[:, 128 * c:128 * (c + 1)],
                                         h2[:, 512 * j:512 * (j + 1)], start=True, stop=True)
                        if j % 2 == 0:
                            nc.scalar.activation(h3t[:, 512 * j:512 * (j + 1)], p3[:], AF.Relu,
                                                 bias=b3[:, c:c + 1])
                        else:
                            nc.vector.tensor_scalar(
                                out=h3t[:, 512 * j:512 * (j + 1)], in0=p3[:],
                                scalar1=b3[:, c:c + 1], scalar2=0.0,
                                op0=OP.add, op1=OP.max)
                # L4 point-major + accum
                for t in range(NCH):
                    m = t // 8
                    lo = 128 * t
                    p4 = pt.tile([128, 384], F32, tag="roll")
                    nc.tensor.matmul(p4[:], h3a[:, lo:lo + 128],
                                     w4a_t[:, 0, :], start=True, stop=False)
                    nc.tensor.matmul(p4[:], h3b[:, lo:lo + 128],
                                     w4a_t[:, 1, :], start=False, stop=False)
                    nc.tensor.matmul(p4[:], xdec[32 * m:32 * m + 4, 128 * (t % 8):128 * (t % 8) + 128],
                                     w4b[32 * m:32 * m + 4, :], start=False, stop=True,
                                     tile_position=(32 * m, 0))
                    fpm = tr.tile([128, 384], BF16, tag="fpm")
                    if t % 2 == 0:
                        nc.scalar.activation(fpm[:], p4[:], AF.Relu)
                    else:
                        nc.vector.tensor_scalar_max(fpm[:], p4[:], 0.0)
                    if k == 0 and t == 0:
                        nc.scalar.copy(fb_row[:], fpm[0:1, :])
                    for c in range(3):
                        nc.tensor.matmul(acc[c][:], fpm[:, 128 * c:128 * (c + 1)],
                                         masks[:, 32 * k + t, :],
                                         start=(k == 0 and t == 0),
                                         stop=(k == 2 and t == NCH - 1))

            # ------------- P2.5: final_in -------------
            fb_ps = pt.tile([128, 4], F32, tag="roll")
            for c in range(3):
                nc.tensor.transpose(fb_ps[:, c:c + 1], fb_row[:, 128 * c:128 * (c + 1)],
                                    ident[0:1, 0:1])
            fb_col = per.tile([128, 4], F32)
            nc.vector.tensor_copy(fb_col[:], fb_ps[:])
            fin = [per.tile([128, 256], BF16, tag=f"fin{c}", name=f"fin{c}") for c in range(3)]
            for c in range(3):
                nc.vector.tensor_copy(fin[c][:], fb_col[:, c:c + 1].to_broadcast([128, 256]))
                nc.vector.copy_predicated(fin[c][:], hasp[:], acc[c][:])

            # ------------- P3: resnet2 + final max -------------
            h1f = [per.tile([128, 256], BF16, tag=f"h1f{m}", name=f"h1f{m}") for m in range(4)]
            for m in range(4):
                pf = pt.tile([128, 256], F32, tag="roll")
                for c in range(3):
                    nc.tensor.matmul(pf[:], g1[:, c, 128 * m:128 * (m + 1)], fin[c][:],
                                     start=(c == 0), stop=False)
                nc.tensor.matmul(pf[:], g1[0:4, 3, 128 * m:128 * (m + 1)], mean_bf[:],
                                 start=False, stop=True)
                nc.scalar.activation(h1f[m][:], pf[:], AF.Relu)
            h2f = [per.tile([128, 256], BF16, tag=f"h2f{m}", name=f"h2f{m}") for m in range(4)]
            for m in range(4):
                pf = pt.tile([128, 256], F32, tag="roll")
                for c in range(4):
                    nc.tensor.matmul(pf[:], g2[:, c, 128 * m:128 * (m + 1)], h1f[c][:],
                                     start=(c == 0), stop=(c == 3))
                if m % 2 == 0:
                    nc.scalar.activation(h2f[m][:], pf[:], AF.Relu, bias=bg2[:, m:m + 1])
                else:
                    nc.vector.tensor_scalar(out=h2f[m][:], in0=pf[:],
                                            scalar1=bg2[:, m:m + 1], scalar2=0.0,
                                            op0=OP.add, op1=OP.max)
            h3f = [per.tile([128, 256], BF16, tag=f"h3f{m}", name=f"h3f{m}") for m in range(6)]
            for m in range(6):
                pf = pt.tile([128, 256], F32, tag="roll")
                for c in range(4):
                    nc.tensor.matmul(pf[:], g3[:, c, 128 * m:128 * (m + 1)], h2f[c][:],
                                     start=(c == 0), stop=(c == 3))
                if m % 2 == 0:
                    nc.scalar.activation(h3f[m][:], pf[:], AF.Relu, bias=bg3[:, m:m + 1])
                else:
                    nc.vector.tensor_scalar(out=h3f[m][:], in0=pf[:],
                                            scalar1=bg3[:, m:m + 1], scalar2=0.0,
                                            op0=OP.add, op1=OP.max)
            outcol = per.tile([128, 8], F32)
            for m in range(8):
                pf = pt.tile([128, 256], F32, tag="roll")
                for c in range(6):
                    nc.tensor.matmul(pf[:], g4[:, c, 128 * m:128 * (m + 1)], h3f[c][:],
                                     start=(c == 0), stop=False)
                for c in range(3):
                    nc.tensor.matmul(pf[:], g4[:, 6 + c, 128 * m:128 * (m + 1)], fin[c][:],
                                     start=False, stop=False)
                nc.tensor.matmul(pf[:], g4[0:4, 9, 128 * m:128 * (m + 1)], mean_bf[:],
                                 start=False, stop=True)
                nc.vector.tensor_reduce(out=outcol[:, m:m + 1], in_=pf[:],
                                        axis=mybir.AxisListType.X, op=OP.max)
            nc.vector.tensor_scalar_max(outcol[:], outcol[:], 0.0)
            nc.sync.dma_start(bass.AP(out_d, 0, [[1, 128], [128, 8]]), outcol[:])

    nc.finalize()
    return nc


def _prep_shared(inputs):
    bf = lambda a: np.ascontiguousarray(np.asarray(a, np.float32)).astype(ml_dtypes.bfloat16)
    f32 = lambda a: np.ascontiguousarray(np.asarray(a, np.float32))
    d = {}
    d["w1"] = bf(np.concatenate([inputs["fp_w0"].T, inputs["fp_b0"][None, :]], 0))
    d["w2"] = bf(np.concatenate([inputs["fp_w1"].T, inputs["fp_b1"][None, :]], 0))
    d["w3"] = bf(inputs["fp_w2"].T)
    d["b3"] = f32(np.asarray(inputs["fp_b2"]).reshape(2, 128).T)
    w4t = np.asarray(inputs["fp_w3"]).T  # [259, 384]
    d["w4a"] = bf(w4t[:256])
    d["w4b"] = bf(np.concatenate([w4t[256:259], inputs["fp_b3"][None, :]], 0))
    # fn stack, final_in reordered to [masked(384); mean(3)]
    g1t = np.asarray(inputs["fn_w0"]).T          # [387, 512]; rows 0:3 mean, 3:387 masked
    g1r = np.concatenate([g1t[3:387], g1t[0:3], np.asarray(inputs["fn_b0"])[None, :]], 0)
    d["g1"] = bf(g1r)                            # [388, 512]
    d["g2"] = bf(np.asarray(inputs["fn_w1"]).T)  # [512, 512]
    d["bg2"] = f32(np.asarray(inputs["fn_b1"]).reshape(4, 128).T)
    d["g3"] = bf(np.asarray(inputs["fn_w2"]).T)  # [512, 768]
    d["bg3"] = f32(np.asarray(inputs["fn_b2"]).reshape(6, 128).T)
    g4t = np.asarray(inputs["fn_w3"]).T          # [1155, 1024]: rows 0:768 h3, 768:771 mean, 771:1155 masked
    g4r = np.concatenate([g4t[0:768], g4t[771:1155], g4t[768:771],
                          np.asarray(inputs["fn_b3"])[None, :]], 0)
    d["g4"] = bf(g4r)                            # [1156, 1024]
    return d


def _prep_node2(node_b):
    n2 = (node_b * node_b).sum(0)
    return np.ascontiguousarray(
        np.concatenate([2.0 * node_b, -n2[None, :]], 0).astype(np.float32))


def kernel(**inputs):
    key = "nc"
    if key not in _CACHE:
        _CACHE[key] = _build()
    nc = _CACHE[key]
    shared = _prep_shared(inputs)
    x = np.ascontiguousarray(np.asarray(inputs["x"], np.float32))
    node = np.ascontiguousarray(np.asarray(inputs["node"], np.float32))
    in_maps = []
    for b in range(B):
        m = dict(shared)
        m["x4"] = np.ascontiguousarray(
            np.concatenate([x[b], np.ones((1, N), np.float32)], 0))
        m["node2"] = _prep_node2(node[b])
        in_maps.append(m)
    res = run_bass_kernel_spmd(nc, in_maps, core_ids=list(range(B)))
    return np.stack([res.results[b]["out"] for b in range(B)], 0)


# revision 27
# speedup vs baseline: 1.0133x; 1.0133x over previous
"""SO-Net encoder kernel for 8 NeuronCores (Bass/Tile), data-parallel over batch."""

import numpy as np
import ml_dtypes

import concourse.bass as bass
import concourse.mybir as mybir
import concourse.tile as tile
from concourse import bacc
from concourse.bass_utils import run_bass_kernel_spmd
from concourse.masks import make_identity

F32 = mybir.dt.float32
F32R = mybir.dt.float32r
BF16 = mybir.dt.bfloat16
U16 = mybir.dt.uint16
I32 = mybir.dt.int32
AF = mybir.ActivationFunctionType
OP = mybir.AluOpType

B, N, M, K = 8, 4096, 256, 3
NCH = N // 128          # 32 point chunks
NB = 4                  # gather blocks (cores 0,2,4,6), 1024 points each
BLK = N // NB           # 1024

_CACHE = {}


def _build():
    nc = bacc.Bacc("TRN2", target_bir_lowering=False)

    # ---------------- DRAM tensors ----------------
    x_d = nc.dram_tensor("x4", [4, N], F32, kind="ExternalInput")
    node2_d = nc.dram_tensor("node2", [4, M], F32, kind="ExternalInput")
    w1_d = nc.dram_tensor("w1", [4, 64], BF16, kind="ExternalInput")     # [fpw0.T; b0]
    w2_d = nc.dram_tensor("w2", [65, 128], BF16, kind="ExternalInput")   # [fpw1.T; b1]
    w3_d = nc.dram_tensor("w3", [128, 256], BF16, kind="ExternalInput")  # fpw2.T
    b3_d = nc.dram_tensor("b3", [128, 2], F32, kind="ExternalInput")     # fp_b2 chunks
    w4a_d = nc.dram_tensor("w4a", [256, 384], BF16, kind="ExternalInput")  # fpw3.T[:256]
    w4b_d = nc.dram_tensor("w4b", [4, 384], BF16, kind="ExternalInput")    # [fpw3.T[256:259]; b3]
    g1_d = nc.dram_tensor("g1", [388, 512], BF16, kind="ExternalInput")
    g2_d = nc.dram_tensor("g2", [512, 512], BF16, kind="ExternalInput")
    bg2_d = nc.dram_tensor("bg2", [128, 4], F32, kind="ExternalInput")
    g3_d = nc.dram_tensor("g3", [512, 768], BF16, kind="ExternalInput")
    bg3_d = nc.dram_tensor("bg3", [128, 6], F32, kind="ExternalInput")
    g4_d = nc.dram_tensor("g4", [1156, 1024], BF16, kind="ExternalInput")
    out_d = nc.dram_tensor("out", [1024], F32, kind="ExternalOutput")
    stage_d = nc.dram_tensor("stage", [3 * N], BF16)  # scratch: flat idx rows

    with tile.TileContext(nc) as tc:
        with (
            tc.tile_pool(name="per", bufs=1) as per,          # persistent sbuf
            tc.tile_pool(name="tr", bufs=5) as tr,            # transient sbuf
            tc.tile_pool(name="tr2", bufs=2) as tr2,          # big transients
            tc.tile_pool(name="pp", bufs=1, space="PSUM") as pp,   # persistent psum
            tc.tile_pool(name="pt", bufs=6, space="PSUM") as pt,   # rolling psum
        ):
            # ------------- constants -------------
            ident = per.tile([128, 128], F32)
            make_identity(nc, ident[:])
            io = per.tile([128, 256], I32)
            nc.gpsimd.iota(io[:], pattern=[[1, 256]], base=0, channel_multiplier=0)
            iota256 = per.tile([128, 256], BF16)
            nc.vector.tensor_copy(iota256[:], io[:])
            e4i = per.tile([4, 32, 4], I32)
            nc.gpsimd.iota(e4i[:], pattern=[[0, 32], [1, 4]], base=0,
                           channel_multiplier=-1)
            e4f = per.tile([4, 128], F32)
            nc.vector.tensor_copy(e4f[:], e4i[:].rearrange("c a b -> c (a b)"))
            e4 = per.tile([4, 128], F32)
            nc.vector.tensor_scalar(out=e4[:], in0=e4f[:], scalar1=0.0, scalar2=None,
                                    op0=OP.is_equal)
            ones_col = per.tile([1, 128], F32)
            nc.vector.memset(ones_col[:], 1.0)
            sel4 = per.tile([4, 1], F32)
            nc.vector.tensor_scalar(out=sel4[:], in0=e4f[:, 3:4], scalar1=0.0,
                                    scalar2=None, op0=OP.is_equal)

            # ------------- weights to SBUF -------------
            w1 = per.tile([128, 64], BF16)      # replicated at 32-offsets
            w4b = per.tile([128, 384], BF16)
            for m in range(4):
                nc.sync.dma_start(w1[32 * m:32 * m + 4, :], w1_d[:])
                nc.sync.dma_start(w4b[32 * m:32 * m + 4, :], w4b_d[:])
            w2 = per.tile([65, 128], BF16)
            nc.sync.dma_start(w2[:], w2_d[:])
            w3 = per.tile([128, 256], BF16)
            nc.sync.dma_start(w3[:], w3_d[:])
            b3 = per.tile([128, 2], F32)
            nc.sync.dma_start(b3[:], b3_d[:])
            g1 = per.tile([128, 4, 512], BF16)   # chunks 0..2 = 128 rows, chunk 3 rows 0..3
            g2 = per.tile([128, 4, 512], BF16)
            g3 = per.tile([128, 4, 768], BF16)
            g4 = per.tile([128, 10, 1024], BF16)
            bg2 = per.tile([128, 4], F32)
            bg3 = per.tile([128, 6], F32)
            nc.sync.dma_start(bg2[:], bg2_d[:])
            nc.sync.dma_start(bg3[:], bg3_d[:])
            for c in range(3):
                nc.sync.dma_start(g1[:, c, :], g1_d[128 * c:128 * (c + 1), :])
            nc.sync.dma_start(g1[0:4, 3, :], g1_d[384:388, :])
            for c in range(4):
                nc.sync.dma_start(g2[:, c, :], g2_d[128 * c:128 * (c + 1), :])
                nc.sync.dma_start(g3[:, c, :], g3_d[128 * c:128 * (c + 1), :])
            for c in range(9):
                nc.sync.dma_start(g4[:, c, :], g4_d[128 * c:128 * (c + 1), :])
            nc.sync.dma_start(g4[0:4, 9, :], g4_d[1152:1156, :])
            w4a_t = per.tile([128, 2, 384], BF16)
            for c in range(2):
                nc.sync.dma_start(w4a_t[:, c, :], w4a_d[128 * c:128 * (c + 1), :])

            # ------------- x / node prep -------------
            x_sb = per.tile([4, N], F32)
            nc.sync.dma_start(x_sb[:], x_d[:])
            node2 = per.tile([4, M], F32)
            nc.sync.dma_start(node2[:], node2_d[:])

            # x_rep [128, N] f32: row p = x_sb[p%4]
            x_rep = per.tile([128, N], F32)
            for j in range(8):
                xr_ps = pt.tile([128, 512], F32, tag="roll")
                nc.tensor.matmul(xr_ps[:], e4[:], x_sb[:, 512 * j:512 * (j + 1)],
                                 start=True, stop=True)
                nc.scalar.copy(x_rep[:, 512 * j:512 * (j + 1)], xr_ps[:])

            # xT (bf16) [128, 4*NCH]: chunk t cols 4t..4t+4 = x_ext.T rows
            xt_ps = pt.tile([128, 128], F32, tag="roll")
            for t in range(NCH):
                nc.tensor.transpose(xt_ps[:, 4 * t:4 * (t + 1)],
                                    x_sb[:, 128 * t:128 * (t + 1)], ident[0:4, 0:4])
            xT = per.tile([128, 128], BF16)
            nc.vector.tensor_copy(xT[:], xt_ps[:])

            # ------------- P1: scores, top-3, masks, xsums -------------
            masks = per.tile([128, 96, 256], BF16)
            idxflat = per.tile([3, N], BF16)
            bankA = pp.tile([128, 512], F32)
            bankB = pp.tile([128, 512], F32)
            sums_ps = bankB[0:4, 256:512]
            for t in range(NCH):
                s_ps = pt.tile([128, 256], F32, tag="roll")
                nc.tensor.matmul(s_ps[:], x_sb[:, 128 * t:128 * (t + 1)], node2[:],
                                 start=True, stop=True)
                s_sb = tr.tile([128, 256], F32, tag="s_sb")
                nc.scalar.copy(s_sb[:], s_ps[:])
                mx8 = tr.tile([128, 8], F32, tag="mx8")
                nc.vector.max(out=mx8[:], in_=s_sb[:])
                idx8 = tr.tile([128, 8], U16, tag="idx8")
                nc.vector.max_index(out=idx8[:], in_max=mx8[:], in_values=s_sb[:])
                idxf = tr.tile([128, 8], F32, tag="idxf")
                nc.scalar.copy(idxf[:], idx8[:])
                # masks for k=0,1,2 (split across DVE and GpSimd)
                meng = nc.vector if t % 2 == 0 else nc.gpsimd
                for k in range(3):
                    meng.tensor_scalar(out=masks[:, 32 * k + t, :], in0=iota256[:],
                                       scalar1=idxf[:, k:k + 1], scalar2=None,
                                       op0=OP.is_equal)
                    nc.tensor.matmul(sums_ps, xT[:, 4 * t:4 * (t + 1)],
                                     masks[:, 32 * k + t, :],
                                     start=(t == 0 and k == 0),
                                     stop=(t == NCH - 1 and k == 2))
                # transpose idx -> flat (pack 4 chunks per psum tile)
                if t % 4 == 0:
                    tp_ps = pt.tile([8, 512], F32, tag="roll")
                nc.tensor.transpose(tp_ps[:, 128 * (t % 4):128 * (t % 4 + 1)],
                                    idxf[:], ident[:])
                if t % 4 == 3:
                    nc.scalar.copy(idxflat[:, 128 * (t - 3):128 * (t + 1)], tp_ps[0:3, :])

            # ------------- P1.5: mean tables -------------
            sums_sb = per.tile([4, 256], F32)
            nc.vector.tensor_copy(sums_sb[:], sums_ps)
            cnt_ps = pt.tile([1, 256], F32, tag="roll")
            nc.tensor.matmul(cnt_ps[:], sel4[:], sums_sb[:], start=True, stop=True)
            cnt_row = per.tile([1, 256], F32)
            nc.vector.tensor_copy(cnt_row[:], cnt_ps[:])
            rec = per.tile([1, 256], F32)
            nc.vector.tensor_scalar_add(rec[:], cnt_row[:], 1e-5)
            nc.vector.reciprocal(rec[:], rec[:])
            rr_ps = pt.tile([4, 256], F32, tag="roll")
            nc.tensor.matmul(rr_ps[:], ones_col[0:1, 0:4], rec[:], start=True, stop=True)
            rec4 = per.tile([4, 256], F32)
            nc.vector.tensor_copy(rec4[:], rr_ps[:])
            mean_sb = per.tile([4, 256], F32)
            nc.vector.memset(mean_sb[:], 0.0)
            nc.vector.tensor_tensor(out=mean_sb[0:3, :], in0=sums_sb[0:3, :],
                                    in1=rec4[0:3, :], op=OP.mult)
            mr_ps = pt.tile([128, 256], F32, tag="roll")
            nc.tensor.matmul(mr_ps[:], e4[:], mean_sb[:], start=True, stop=True)
            mean_rep = per.tile([128, 256], F32)
            nc.vector.tensor_copy(mean_rep[:], mr_ps[:])
            mean_bf = per.tile([4, 256], BF16)
            nc.vector.memset(mean_bf[:], 1.0)
            nc.vector.tensor_copy(mean_bf[0:3, :], mean_sb[0:3, :])
            hp_ps = pt.tile([128, 256], F32, tag="roll")
            nc.tensor.matmul(hp_ps[:], ones_col[:], cnt_row[:], start=True, stop=True)
            hasp = per.tile([128, 256], I32)
            nc.vector.tensor_copy(hasp[:], hp_ps[:])
            nc.sync.dma_start(stage_d[:], idxflat[:])

            # ------------- P2: per-k resnet1 + accumulation -------------
            acc = [bankA[:, 0:256], bankA[:, 256:512], bankB[:, 0:256]]
            fb_row = per.tile([1, 384], F32)
            h1 = per.tile([65, N], BF16)
            nc.vector.memset(h1[64:65, :], 1.0)
            h2 = per.tile([128, N], BF16)
            h3a = per.tile([128, N], BF16)
            h3b = per.tile([128, N], BF16)
            for k in range(3):
                wrapf = tr.tile([128, BLK // 16], BF16, tag="wrapf")
                nc.vector.memset(wrapf[:], 0.0)
                for m in range(NB):
                    nc.sync.dma_start(
                        wrapf[32 * m:32 * m + 16, :],
                        bass.AP(stage_d, k * N + BLK * m, [[1, 16], [16, BLK // 16]]))
                wrap = tr.tile([128, BLK // 16], U16, tag="wrap")
                nc.vector.tensor_copy(wrap[:], wrapf[:])
                ctr = tr2.tile([128, BLK, 1], F32, tag="ctr")
                nc.gpsimd.indirect_copy(ctr[:], mean_rep[:].rearrange("p (a b) -> p a b", b=1),
                                        wrap[:], True)
                xdec = tr2.tile([128, BLK], BF16, tag="xdec")
                for m in range(NB):
                    nc.vector.tensor_tensor(
                        out=xdec[32 * m:32 * m + 16, :],
                        in0=x_rep[32 * m:32 * m + 16, BLK * m:BLK * (m + 1)],
                        in1=ctr[32 * m:32 * m + 16, :, 0], op=OP.subtract)
                # L1: [64, N] bf16 (+ones row)
                for m in range(NB):
                    for h in range(2):
                        p1 = pt.tile([64, 512], F32, tag="roll")
                        nc.tensor.matmul(p1[:], w1[32 * m:32 * m + 4, :],
                                         xdec[32 * m:32 * m + 4, 512 * h:512 * (h + 1)],
                                         start=True, stop=True,
                                         tile_position=(32 * m, 0))
                        if (2 * m + h) % 2 == 0:
                            nc.scalar.activation(h1[0:64, BLK * m + 512 * h:BLK * m + 512 * (h + 1)],
                                                 p1[:], AF.Relu)
                        else:
                            nc.vector.tensor_scalar_max(
                                h1[0:64, BLK * m + 512 * h:BLK * m + 512 * (h + 1)],
                                p1[:], 0.0)
                # L2: [128, N]
                for j in range(8):
                    p2 = pt.tile([128, 512], F32, tag="roll")
                    nc.tensor.matmul(p2[:], w2[:], h1[:, 512 * j:512 * (j + 1)],
                                     start=True, stop=True)
                    if j % 2 == 0:
                        nc.scalar.activation(h2[:, 512 * j:512 * (j + 1)], p2[:], AF.Relu)
                    else:
                        nc.vector.tensor_scalar_max(h2[:, 512 * j:512 * (j + 1)], p2[:], 0.0)
                # L3: [256 -> 2 tiles, N], bias via ACT
                for c, h3t in enumerate([h3a, h3b]):
                    for j in range(8):
                        p3 = pt.tile([128, 512], F32, tag="roll")
                        nc.tensor.matmul(p3[:], w3[:, 128 * c:128 * (c + 1)],
                                         h2[:, 512 * j:512 * (j + 1)], start=True, stop=True)
                        if j % 2 == 0:
                            nc.scalar.activation(h3t[:, 512 * j:512 * (j + 1)], p3[:], AF.Relu,
                                                 bias=b3[:, c:c + 1])
                        else:
                            nc.vector.tensor_scalar(
                                out=h3t[:, 512 * j:512 * (j + 1)], in0=p3[:],
                                scalar1=b3[:, c:c + 1], scalar2=0.0,
                                op0=OP.add, op1=OP.max)
                # L4 point-major + accum
                for t in range(NCH):
                    m = t // 8
                    lo = 128 * t
                    p4 = pt.tile([128, 384], F32, tag="roll")
                    nc.tensor.matmul(p4[:], h3a[:, lo:lo + 128],
                                     w4a_t[:, 0, :], start=True, stop=False)
                    nc.tensor.matmul(p4[:], h3b[:, lo:lo + 128],
                                     w4a_t[:, 1, :], start=False, stop=False)
                    nc.tensor.matmul(p4[:], xdec[32 * m:32 * m + 4, 128 * (t % 8):128 * (t % 8) + 128],
                                     w4b[32 * m:32 * m + 4, :], start=False, stop=True,
                                     tile_position=(32 * m, 0))
                    fpm = tr.tile([128, 384], BF16, tag="fpm")
                    if t % 2 == 0:
                        nc.scalar.activation(fpm[:], p4[:], AF.Relu)
                    else:
                        nc.vector.tensor_scalar_max(fpm[:], p4[:], 0.0)
                    if k == 0 and t == 0:
                        nc.scalar.copy(fb_row[:], fpm[0:1, :])
                    for c in range(3):
                        nc.tensor.matmul(acc[c], fpm[:, 128 * c:128 * (c + 1)],
                                         masks[:, 32 * k + t, :],
                                         start=(k == 0 and t == 0),
                                         stop=(k == 2 and t == NCH - 1))

            # ------------- P2.5: final_in -------------
            fb_ps = pt.tile([128, 4], F32, tag="roll")
            for c in range(3):
                nc.tensor.transpose(fb_ps[:, c:c + 1], fb_row[:, 128 * c:128 * (c + 1)],
                                    ident[0:1, 0:1])
            fb_col = per.tile([128, 4], F32)
            nc.vector.tensor_copy(fb_col[:], fb_ps[:])
            fin = [per.tile([128, 256], BF16, tag=f"fin{c}", name=f"fin{c}") for c in range(3)]
            for c in range(3):
                nc.vector.tensor_copy(fin[c][:], fb_col[:, c:c + 1].to_broadcast([128, 256]))
                nc.vector.copy_predicated(fin[c][:], hasp[:], acc[c])

            # ------------- P3: resnet2 + final max -------------
            h1f = [per.tile([128, 256], BF16, tag=f"h1f{m}", name=f"h1f{m}") for m in range(4)]
            for m in range(4):
                pf = pt.tile([128, 256], F32, tag="roll")
                for c in range(3):
                    nc.tensor.matmul(pf[:], g1[:, c, 128 * m:128 * (m + 1)], fin[c][:],
                                     start=(c == 0), stop=False)
                nc.tensor.matmul(pf[:], g1[0:4, 3, 128 * m:128 * (m + 1)], mean_bf[:],
                                 start=False, stop=True)
                nc.scalar.activation(h1f[m][:], pf[:], AF.Relu)
            h2f = [per.tile([128, 256], BF16, tag=f"h2f{m}", name=f"h2f{m}") for m in range(4)]
            for m in range(4):
                pf = pt.tile([128, 256], F32, tag="roll")
                for c in range(4):
                    nc.tensor.matmul(pf[:], g2[:, c, 128 * m:128 * (m + 1)], h1f[c][:],
                                     start=(c == 0), stop=(c == 3))
                if m % 2 == 0:
                    nc.scalar.activation(h2f[m][:], pf[:], AF.Relu, bias=bg2[:, m:m + 1])
                else:
                    nc.vector.tensor_scalar(out=h2f[m][:], in0=pf[:],
                                            scalar1=bg2[:, m:m + 1], scalar2=0.0,
                                            op0=OP.add, op1=OP.max)
            h3f = [per.tile([128, 256], BF16, tag=f"h3f{m}", name=f"h3f{m}") for m in range(6)]
            for m in range(6):
                pf = pt.tile([128, 256], F32, tag="roll")
                for c in range(4):
                    nc.tensor.matmul(pf[:], g3[:, c, 128 * m:128 * (m + 1)], h2f[c][:],
                                     start=(c == 0), stop=(c == 3))
                if m % 2 == 0:
                    nc.scalar.activation(h3f[m][:], pf[:], AF.Relu, bias=bg3[:, m:m + 1])
                else:
                    nc.vector.tensor_scalar(out=h3f[m][:], in0=pf[:],
                                            scalar1=bg3[:, m:m + 1], scalar2=0.0,
                                            op0=OP.add, op1=OP.max)
            outcol = per.tile([128, 8], F32)
            for m in range(8):
                pf = pt.tile([128, 256], F32, tag="roll")
                for c in range(6):
                    nc.tensor.matmul(pf[:], g4[:, c, 128 * m:128 * (m + 1)], h3f[c][:],
                                     start=(c == 0), stop=False)
                for c in range(3):
                    nc.tensor.matmul(pf[:], g4[:, 6 + c, 128 * m:128 * (m + 1)], fin[c][:],
                                     start=False, stop=False)
                nc.tensor.matmul(pf[:], g4[0:4, 9, 128 * m:128 * (m + 1)], mean_bf[:],
                                 start=False, stop=True)
                nc.vector.tensor_reduce(out=outcol[:, m:m + 1], in_=pf[:],
                                        axis=mybir.AxisListType.X, op=OP.max)
            nc.vector.tensor_scalar_max(outcol[:], outcol[:], 0.0)
            nc.sync.dma_start(bass.AP(out_d, 0, [[1, 128], [128, 8]]), outcol[:])

    nc.finalize()
    return nc


def _prep_shared(inputs):
    bf = lambda a: np.ascontiguousarray(np.asarray(a, np.float32)).astype(ml_dtypes.bfloat16)
    f32 = lambda a: np.ascontiguousarray(np.asarray(a, np.float32))
    d = {}
    d["w1"] = bf(np.concatenate([inputs["fp_w0"].T, inputs["fp_b0"][None, :]], 0))
    d["w2"] = bf(np.concatenate([inputs["fp_w1"].T, inputs["fp_b1"][None, :]], 0))
    d["w3"] = bf(inputs["fp_w2"].T)
    d["b3"] = f32(np.asarray(inputs["fp_b2"]).reshape(2, 128).T)
    w4t = np.asarray(inputs["fp_w3"]).T  # [259, 384]
    d["w4a"] = bf(w4t[:256])
    d["w4b"] = bf(np.concatenate([w4t[256:259], inputs["fp_b3"][None, :]], 0))
    # fn stack, final_in reordered to [masked(384); mean(3)]
    g1t = np.asarray(inputs["fn_w0"]).T          # [387, 512]; rows 0:3 mean, 3:387 masked
    g1r = np.concatenate([g1t[3:387], g1t[0:3], np.asarray(inputs["fn_b0"])[None, :]], 0)
    d["g1"] = bf(g1r)                            # [388, 512]
    d["g2"] = bf(np.asarray(inputs["fn_w1"]).T)  # [512, 512]
    d["bg2"] = f32(np.asarray(inputs["fn_b1"]).reshape(4, 128).T)
    d["g3"] = bf(np.asarray(inputs["fn_w2"]).T)  # [512, 768]
    d["bg3"] = f32(np.asarray(inputs["fn_b2"]).reshape(6, 128).T)
    g4t = np.asarray(inputs["fn_w3"]).T          # [1155, 1024]: rows 0:768 h3, 768:771 mean, 771:1155 masked
    g4r = np.concatenate([g4t[0:768], g4t[771:1155], g4t[768:771],
                          np.asarray(inputs["fn_b3"])[None, :]], 0)
    d["g4"] = bf(g4r)                            # [1156, 1024]
    return d


def _prep_node2(node_b):
    n2 = (node_b * node_b).sum(0)
    return np.ascontiguousarray(
        np.concatenate([2.0 * node_b, -n2[None, :]], 0).astype(np.float32))


def kernel(**inputs):
    key = "nc"
    if key not in _CACHE:
        _CACHE[key] = _build()
    nc = _CACHE[key]
    shared = _prep_shared(inputs)
    x = np.ascontiguousarray(np.asarray(inputs["x"], np.float32))
    node = np.ascontiguousarray(np.asarray(inputs["node"], np.float32))
    in_maps = []
    for b in range(B):
        m = dict(shared)
        m["x4"] = np.ascontiguousarray(
            np.concatenate([x[b], np.ones((1, N), np.float32)], 0))
        m["node2"] = _prep_node2(node[b])
        in_maps.append(m)
    res = run_bass_kernel_spmd(nc, in_maps, core_ids=list(range(B)))
    return np.stack([res.results[b]["out"] for b in range(B)], 0)
